# revision 2
# baseline (speedup 1.0000x reference)
"""Bass/Trainium2 kernel for nn_DiffAllocator (64x7 Sinkhorn, 200 iterations).

Algorithm: the reference runs 200 log-domain Sinkhorn iterations. Iteration 1
(the only exact-LSE user) runs on host; iterations 2..200 run on device in a
multiplicative form anchored at stabilizers, re-anchored ("fold") every W=16
iterations so no flushed-to-zero entry is ever remembered (same scheme as the
previous kernel revision).

Per-iteration device structure (the critical path):
  u-half on PE:  r_lo = A1[:,0:32]^T q ; r_hi = A1[:,32:64]^T q  (two tiny
                 matmuls into one [32,2] PSUM tile; back-to-back so their
                 fixed SBUF-fetch latencies overlap)
  q-half on DVE: u2 = 1/r2 (two free scalar-AP reciprocals), M2 = A2p * u2
                 (stride-0 broadcast tensor_tensor), s2 = transpose-reduce of
                 M2 [32,2,32] (tensor_reduce apply_transpose sums each 32-col
                 block across partitions), c = s2[:,0]+s2[:,1], q = 1/c.

The q-half replaces the second PE matmul of the previous revision. A PE
matvec costs ~233ns on the dependency chain (29 sem-recv + 173 fixed SBUF
access + 31 sem-send) while the DVE chain costs ~200ns and its reciprocals
ride along for free, cutting the iteration from ~536ns to ~475ns. All DVE ops
are same-engine in program order, so no semaphores are needed inside the
q-half.

Data is packed [32, 2*7] (rows 0:32 | rows 32:64 side by side) so every
operand sits at partition base 0 - the BIR verifier rejects operands at
differing partition bases. The transpose-reduce's two 32-blocks land the lo/hi
column sums on the same partitions as two free-axis columns, making the final
merge a free [32,1]+[32,1] add.

Folds run on Pool/ACT/PE off the critical path in deferred stages; the basis
switch applies DELAY=7 iterations later via the pre-scaled A1sw (GPSIMD
bitcast approx-ln for stabilizers - no ACT Ln table ever loaded).
"""

import numpy as np

L, B = 64, 7
EPS = 0.02
ITERS = 200
W = 16      # fold window
DELAY = 7   # iterations between fold snapshot and basis switch (must be < W)

_CACHE = {}
_DEBUG_MAP = {}


def _dbg(inst, label):
    try:
        _DEBUG_MAP[inst.name] = label
    except Exception:
        pass
    return inst


def _build_nc(reps=1):
    import concourse.bacc as bacc
    import concourse.tile as tile
    import concourse.bass as bass
    import concourse.mybir as mybir

    f32 = mybir.dt.float32
    AF = mybir.ActivationFunctionType
    OP = mybir.AluOpType
    AX = mybir.AxisListType
    MS = bass.MemorySpace

    nc = bacc.Bacc("TRN2", target_bir_lowering=False, debug=False)

    # ---- DRAM I/O ----
    # W7  = [A1_0 (64) | lb | 1/b | b | psi0]          -> [7, 68]
    # W32 = [Kp (14) | A2p_0 (14) | la2 | 1/a2 | pa2_0] -> [32, 34]
    d_W7 = nc.dram_tensor("W7_in", [B, L + 4], f32, kind="ExternalInput").ap()
    d_W32 = nc.dram_tensor("W32_in", [32, 34], f32, kind="ExternalInput").ap()
    d_id = nc.dram_tensor("ident_in", [32, 32], f32, kind="ExternalInput").ap()
    d_P = nc.dram_tensor("P_out", [L, B], f32, kind="ExternalOutput").ap()

    with tile.TileContext(nc) as tc:
        with (
            tc.tile_pool(name="sb", bufs=1) as sb,
            tc.tile_pool(name="ps", bufs=2, space=MS.PSUM) as ps,
        ):
            def t(shape, tag):
                return sb.tile(shape, f32, tag=tag, name=tag)

            W7 = t([B, L + 4], "W7")
            lbc, invb, bcol, psi1 = (W7[:, L + i:L + i + 1] for i in range(4))
            W32 = t([32, 34], "W32")
            Kp = W32[:, 0:14]
            la2 = W32[:, 28:30]
            inva2 = W32[:, 30:32]
            ident = t([32, 32], "ident")
            A1 = [W7[:, 0:L], t([B, L], "A1_1")]
            A2p = [W32[:, 14:28], t([32, 14], "A2p_1")]
            A1sw = t([B, L], "A1sw")
            M2 = t([32, 64], "M2")
            s2 = t([32, 2], "s2")
            cc = t([32, 1], "cc")
            q = [t([B, 1], "q0"), t([B, 1], "q1")]
            u2 = [t([32, 2], "u2_0"), t([32, 2], "u2_1")]
            pa2 = [t([32, 2], "pa2_0"), W32[:, 32:34]]
            psi_c = [t([B, 1], "psiC0"), psi1]
            psi_r = t([1, 14], "psi_r")
            psi_bc = t([32, 14], "psi_bc")
            T1 = t([32, 14], "T1")
            T2 = t([32, 14], "T2")
            X2 = t([32, 14], "X2")
            lnu2 = t([32, 2], "lnu2")
            yfu2 = t([32, 2], "yfu2")
            t32 = t([32, 2], "t32")
            lnq = t([B, 1], "lnq")
            yfq = t([B, 1], "yfq")
            ncb = t([B, 1], "ncb")
            cb = t([B, 1], "cb")
            Pu2 = t([32, 14], "Pu2")
            bq = t([B, 1], "bq")
            PT7 = t([B, L], "PT7")

            # M2 cols 7:32 and 39:64 must be (and stay) zero: the per-iteration
            # writes only touch cols 0:7 and 32:39.
            nc.vector.memset(M2[:], 0)

            # ---- load inputs (two DMA queues in parallel) ----
            nc.sync.dma_start(out=W7[:], in_=d_W7)
            nc.gpsimd.dma_start(out=W32[:], in_=d_W32)
            nc.sync.dma_start(out=ident[:], in_=d_id)

            # dummy Exp: pulls the one exp_and_others table load into the DMA
            # head instead of the first fold's regen chain.
            scr7 = t([B, 1], "scr7")
            nc.scalar.activation(scr7[:], lbc, AF.Exp)

            # GPSIMD approximate ln (bitcast trick): stabilizers only need to
            # be within ~40 of the true log, so max err ~0.03 is fine.
            LN2 = float(np.log(2.0))
            C1, C2 = LN2 / (2.0 ** 23), -127.0 * LN2

            def gps_ln(out_ap, yf_ap, x_ap):
                nc.gpsimd.tensor_copy(yf_ap, x_ap.bitcast(mybir.dt.uint32))
                nc.gpsimd.tensor_scalar(out=out_ap, in0=yf_ap, scalar1=C1,
                                        scalar2=C2, op0=OP.mult, op1=OP.add)

            # ---- iterations 2..200 ----
            epoch = 0
            fold_idx = 0
            switch_at = None
            deferred = {}
            n_iter_end = 2 + (ITERS - 1) * reps
            for it in range(2, n_iter_end):
                par = it % 2
                q_in = invb if it == 2 else q[(it - 1) % 2]
                switching = switch_at == it
                lhs1 = A1sw if switching else A1[epoch]
                a2cur = A2p[1 - epoch] if switching else A2p[epoch]
                psr2 = ps.tile([32, 2], f32, tag="psr", bufs=3)
                _dbg(nc.tensor.matmul(psr2[:, 0:1], lhs1[:, 0:32], q_in[:],
                                      start=True, stop=True), f"it{it}-mmlo")
                _dbg(nc.tensor.matmul(psr2[:, 1:2], lhs1[:, 32:64], q_in[:],
                                      start=True, stop=True), f"it{it}-mmhi")
                _dbg(nc.vector.reciprocal(u2[par][:, 0:1], psr2[:, 0:1]),
                     f"it{it}-ulo")
                _dbg(nc.vector.reciprocal(u2[par][:, 1:2], psr2[:, 1:2]),
                     f"it{it}-uhi")
                m2v = M2[:].rearrange("p (g c) -> p g c", g=2)[:, :, 0:B]
                a2v = a2cur.rearrange("p (g j) -> p g j", g=2)
                u2b = u2[par][:].unsqueeze(2).broadcast_to([32, 2, B])
                _dbg(nc.vector.tensor_tensor(out=m2v, in0=a2v, in1=u2b,
                                             op=OP.mult), f"it{it}-mul")
                m2r = M2[:].rearrange("p (b x) -> p b x", b=2)
                _dbg(nc.vector.tensor_reduce(out=s2[:], in_=m2r, axis=AX.X,
                                             op=OP.add, apply_transpose=True),
                     f"it{it}-trt")
                _dbg(nc.vector.tensor_tensor(out=cc[:], in0=s2[:, 0:1],
                                             in1=s2[:, 1:2], op=OP.add),
                     f"it{it}-add")
                _dbg(nc.vector.reciprocal(q[par][:], cc[0:B, :]), f"it{it}-qrec")
                if switching:
                    epoch = 1 - epoch
                    switch_at = None

                if it % W == 0 and it + DELAY < n_iter_end - 1 and switch_at is None:
                    # fold: snapshot (u2, q) of this iteration; regen A into
                    # the other epoch buffers; switch basis at it+DELAY.
                    ne = 1 - epoch
                    fp = fold_idx % 2
                    gps_ln(lnu2[:], yfu2[:], u2[par][:])
                    gps_ln(lnq[:], yfq[:], q[par][:])
                    # pa2' = lnu2 + la2 + pa2_old ; psi' = lnq + lb + psi_old
                    _dbg(nc.gpsimd.tensor_tensor(out=t32[:], in0=lnu2[:],
                                                 in1=la2, op=OP.add), f"f{it}-t32")
                    _dbg(nc.gpsimd.tensor_tensor(out=pa2[fp][:], in0=t32[:],
                                                 in1=pa2[1 - fp][:], op=OP.add),
                         f"f{it}-pa")
                    _dbg(nc.gpsimd.tensor_scalar(out=psi_c[fp][:], in0=lnq[:],
                                                 scalar1=psi_c[1 - fp][:],
                                                 scalar2=lbc, op0=OP.add,
                                                 op1=OP.add), f"f{it}-psi")
                    _dbg(nc.gpsimd.tensor_scalar(out=ncb[:], in0=lnq[:],
                                                 scalar1=lbc, scalar2=-1.0,
                                                 op0=OP.add, op1=OP.mult),
                         f"f{it}-ncb")
                    _dbg(nc.scalar.activation(cb[:], ncb[:], AF.Exp), f"f{it}-cb")

                    def stage_b(ne=ne, fp=fp, it=it):
                        # A2p' = exp(Kp + pa2 (+)cols  psi (+)rows)
                        psq = ps.tile([1, B], f32, tag="pst", name="psq")
                        _dbg(nc.tensor.transpose(psq[:], psi_c[fp][:],
                                                 ident[0:B, 0:B]), f"f{it}-psiT")
                        _dbg(nc.scalar.copy(psi_r[:],
                                            psq[:].unsqueeze(1).broadcast_to([1, 2, B])),
                             f"f{it}-psirow")
                        _dbg(nc.gpsimd.partition_broadcast(psi_bc[:], psi_r[:]),
                             f"f{it}-bcast")
                        _dbg(nc.gpsimd.tensor_tensor(out=T1[:], in0=Kp,
                                                     in1=psi_bc[:], op=OP.add),
                             f"f{it}-T1")
                        _dbg(nc.gpsimd.tensor_scalar(out=T2[:, 0:B], in0=T1[:, 0:B],
                                                     scalar1=pa2[fp][:, 0:1],
                                                     scalar2=None, op0=OP.add),
                             f"f{it}-T2lo")
                        _dbg(nc.gpsimd.tensor_scalar(out=T2[:, B:2 * B],
                                                     in0=T1[:, B:2 * B],
                                                     scalar1=pa2[fp][:, 1:2],
                                                     scalar2=None, op0=OP.add),
                             f"f{it}-T2hi")
                        _dbg(nc.scalar.activation(A2p[ne][:], T2[:], AF.Exp),
                             f"f{it}-A2exp")

                    def stage_c(ne=ne, it=it):
                        # A1' = (A2' * 1/a)^T * b ; switch matrix A1sw = A1' * cb
                        _dbg(nc.gpsimd.tensor_scalar(out=X2[:, 0:B],
                                                     in0=A2p[ne][:, 0:B],
                                                     scalar1=inva2[:, 0:1],
                                                     scalar2=None, op0=OP.mult),
                             f"f{it}-Xlo")
                        _dbg(nc.gpsimd.tensor_scalar(out=X2[:, B:2 * B],
                                                     in0=A2p[ne][:, B:2 * B],
                                                     scalar1=inva2[:, 1:2],
                                                     scalar2=None, op0=OP.mult),
                             f"f{it}-Xhi")
                        psa = ps.tile([B, L], f32, tag="pst", name="psa")
                        _dbg(nc.tensor.transpose(psa[:, 0:32], X2[:, 0:B],
                                                 ident[:]), f"f{it}-XTlo")
                        _dbg(nc.tensor.transpose(psa[:, 32:64], X2[:, B:2 * B],
                                                 ident[:]), f"f{it}-XThi")
                        _dbg(nc.scalar.activation(A1[ne][:], psa[:], AF.Copy,
                                                  scale=bcol), f"f{it}-A1")
                        _dbg(nc.gpsimd.tensor_scalar(out=A1sw[:], in0=A1[ne][:],
                                                     scalar1=cb[:], scalar2=None,
                                                     op0=OP.mult), f"f{it}-A1sw")

                    deferred.setdefault(it + 3, []).append(stage_b)
                    deferred.setdefault(it + 4, []).append(stage_c)
                    fold_idx += 1
                    switch_at = it + DELAY

                for fn in deferred.pop(it, []):
                    fn()

            # ---- final: P = diag(u) A2 diag(b q), built transposed [7,64] ----
            fpar = (n_iter_end - 1) % 2
            pu2v = Pu2[:].rearrange("p (g j) -> p g j", g=2)
            a2vf = A2p[epoch].rearrange("p (g j) -> p g j", g=2)
            u2bf = u2[fpar][:].unsqueeze(2).broadcast_to([32, 2, B])
            nc.vector.tensor_tensor(out=pu2v, in0=a2vf, in1=u2bf, op=OP.mult)
            nc.vector.tensor_scalar(out=bq[:], in0=q[fpar][:], scalar1=bcol,
                                    scalar2=None, op0=OP.mult)
            psp = ps.tile([B, L], f32, tag="pst")
            nc.tensor.transpose(psp[:, 0:32], Pu2[:, 0:B], ident[:])
            nc.tensor.transpose(psp[:, 32:64], Pu2[:, B:2 * B], ident[:])
            nc.vector.tensor_scalar(out=PT7[:], in0=psp[:], scalar1=bq[:],
                                    scalar2=None, op0=OP.mult)
            nc.sync.dma_start(out=d_P.rearrange("a b -> b a"), in_=PT7[:])

    nc.compile()
    return nc


def _host_inputs(theta, phi, n, sens, err):
    f32 = np.float32
    theta = np.asarray(theta, f32); phi = np.asarray(phi, f32)
    n = np.asarray(n, f32); sens = np.asarray(sens, f32)
    err = np.asarray(err, f32)
    a = (n / n.sum()).astype(f32)
    e = np.exp((phi - phi.max()).astype(f32)); b = (e / e.sum()).astype(f32)
    C = ((n * sens)[:, None] * err[None, :]).astype(f32)
    K = ((theta - C) * f32(1.0 / EPS)).astype(f32)
    la = np.log(a).astype(f32)
    lb = np.log(b).astype(f32)

    # iteration 1 (log domain, max-stabilized LSE) + initial basis, on host
    def lse(x, axis):
        m = x.max(axis=axis, keepdims=True)
        return (m + np.log(np.exp(x - m).sum(axis=axis, keepdims=True))
                ).squeeze(axis).astype(f32)

    def ftz(x):
        x = np.asarray(x, f32).copy()
        x[np.abs(x) < 1.17549435e-38] = 0.0
        return x

    f1 = (la - lse(K, 1)).astype(f32)
    g1 = (lb - lse(K + f1[:, None], 0)).astype(f32)
    pa0 = (f1 + la).astype(f32)
    A2_0 = ftz(np.exp((K + pa0[:, None] + g1[None, :]).astype(f32)))
    A1_0 = ftz(ftz(A2_0 * (f32(1.0) / a)[:, None]).T * b[:, None])

    inva = (f32(1.0) / a).astype(f32)
    pack = lambda x: np.concatenate([x[0:32], x[32:64]], axis=1).astype(f32)
    pack1 = lambda x: np.stack([x[0:32], x[32:64]], axis=1).astype(f32)
    W7 = np.concatenate(
        [A1_0, np.stack([lb, f32(1.0) / b, b, g1], axis=1)], axis=1).astype(f32)
    W32 = np.concatenate(
        [pack(K), pack(A2_0), pack1(la), pack1(inva), pack1(pa0)],
        axis=1).astype(f32)
    return {
        "W7_in": np.ascontiguousarray(W7),
        "W32_in": np.ascontiguousarray(W32),
        "ident_in": np.eye(32, dtype=f32),
    }


def kernel(theta, phi, n, sens, err):
    if "nc" not in _CACHE:
        _CACHE["nc"] = _build_nc()
    nc = _CACHE["nc"]
    in_map = _host_inputs(theta, phi, n, sens, err)
    from concourse import bass_utils
    res = bass_utils.run_bass_kernel_spmd(nc, [in_map], [0])
    return np.asarray(res.results[0]["P_out"], dtype=np.float32)


# revision 23
# speedup vs baseline: 1.5166x; 1.5166x over previous
"""Bass/Trainium2 kernel for nn_DiffAllocator (64x7 Sinkhorn, 200 iterations).

Raw-bass implementation (no Tile framework) with hand-rolled semaphores,
using the proven 4-op loop structure:

    r = A1 q   (PE matvec)   u = 1/r  (DVE reciprocal)
    c = A2 u   (PE matvec)   q = 1/c  (DVE reciprocal)

Iteration 1 (the only exact-LSE user) runs on host; iterations 2..200 run on
device in this multiplicative form anchored at stabilizers, re-anchored
("fold") every 16 iterations so no flushed-to-zero entry is ever remembered.
Every in-loop dependency is cross-engine (PE<->DVE semaphores) - the pattern
whose 536ns/iteration cost is hardware-validated. Same-engine DVE chaining is
NOT used (DVE SBUF writes drain ~60-125ns after the engine frees; a program-
order consumer races, as measured).

Why raw bass: the Tile framework routes fold work through the same per-engine
semaphore counters as the loop, so each fold's snapshot reads and PE
transposes stall the loop ~460ns (3 stalls/fold, ~5.6us total). With separate
semaphores per producer/consumer pair the folds run entirely in the loop's
shadow on Pool/ACT (+2 donated PE transpose slots), and the switch waits are
pre-satisfied. The it=192 fold is dropped (the 24-iteration tail drifts only
~e^30, far inside fp32 range) and the basis switch applies DELAY=8 iterations
after the snapshot (fp32-validated in emulation).

The output DMA instruction is issued up-front with its semaphore wait
attached, so the DGE setup overlaps the loop and only the transfer itself
lands in the tail.
"""

import numpy as np

L, B = 64, 7
EPS = 0.02
ITERS = 200
FOLD_EVERY = 16
LAST_FOLD = 176
DELAY = 8       # fold snapshot -> basis switch
PSIT_SLOT = 2   # fold + n: PE emits the psi transpose after this iteration
XT_SLOT = 6     # fold + n: PE emits the X transpose after this iteration

_CACHE = {}


def _build_nc(reps=1):
    import contextlib
    import concourse.bacc as bacc
    import concourse.mybir as mybir

    f32 = mybir.dt.float32
    u32 = mybir.dt.uint32
    AF = mybir.ActivationFunctionType
    OP = mybir.AluOpType

    nc = bacc.Bacc("TRN2", target_bir_lowering=False, debug=False)

    # ---- DRAM I/O ----
    # W7   = [A1_0 (64) | lb | 1/b | b | psi0]      -> [7, 68]
    # W64K = [A2_0 (7) | K (7) | la | 1/a | pa0]    -> [64, 17]
    d_W7 = nc.dram_tensor("W7_in", [B, L + 4], f32, kind="ExternalInput").ap()
    d_WK = nc.dram_tensor("W64K_in", [L, 17], f32, kind="ExternalInput").ap()
    d_id = nc.dram_tensor("ident_in", [L, L], f32, kind="ExternalInput").ap()
    d_P = nc.dram_tensor("P_out", [L, B], f32, kind="ExternalOutput").ap()

    n_end = 2 + (ITERS - 1) * reps
    iters = list(range(2, n_end))
    folds = [it for it in iters if it % FOLD_EVERY == 0 and it <= LAST_FOLD]
    fold_of = {it_f: f for f, it_f in enumerate(folds)}
    switch_of = {it_f + DELAY: f for f, it_f in enumerate(folds)}
    psiT_slot = {it_f + PSIT_SLOT: f for f, it_f in enumerate(folds)}
    xt_slot = {it_f + XT_SLOT: f for f, it_f in enumerate(folds)}
    war_slot = {it_f + 2: f for f, it_f in enumerate(folds)}
    n_folds = len(folds)
    final_epoch = n_folds % 2
    final_par = iters[-1] % 2
    k_last = len(iters)

    LN2 = float(np.log(2.0))
    C1, C2 = LN2 / (2.0 ** 23), -127.0 * LN2

    es = contextlib.ExitStack()
    with es:
        sb = lambda name, shape: es.enter_context(
            nc.sbuf_tensor(name, shape, f32))
        W7 = sb("W7", [B, L + 4])
        WK = sb("WK", [L, 17])
        ident = sb("ident", [L, L])
        A1_1 = sb("A1_1", [B, L])
        A2_1 = sb("A2_1", [L, B])
        A1sw = sb("A1sw", [B, L])
        q0t = sb("q0", [B, 1]); q1t = sb("q1", [B, 1])
        u0t = sb("u0", [L, 1]); u1t = sb("u1", [L, 1])
        pa_0 = sb("pa_0", [L, 1])
        psiC0 = sb("psiC0", [B, 1])
        psi_r = sb("psi_r", [1, B])
        psi_bc = sb("psi_bc", [L, B])
        T1 = sb("T1", [L, B])
        X = sb("X", [L, B])
        lnu = sb("lnu", [L, 1]); yfu = sb("yfu", [L, 1])
        lnq = sb("lnq", [B, 1]); yfq = sb("yfq", [B, 1])
        ncb = sb("ncb", [B, 1]); cb = sb("cb", [B, 1])
        Pu = sb("Pu", [L, B])
        bq = sb("bq", [B, 1])
        PT7 = sb("PT7", [B, L])
        scr7 = sb("scr7", [B, 1])

        psr = es.enter_context(nc.psum_tensor("psr", [L, 1], f32))
        psc = es.enter_context(nc.psum_tensor("psc", [B, 1], f32))
        psq = es.enter_context(nc.psum_tensor("psq", [1, B], f32))
        psa = es.enter_context(nc.psum_tensor("psa", [B, L], f32))

        sem = lambda name: es.enter_context(nc.semaphore(name))
        dsem = sem("dsem")          # W7 DMA
        dsemI = sem("dsemI")        # ident DMA
        dsem2 = sem("dsem2")        # W64K DMA (Pool queue)
        pe_sem = sem("pe_sem")      # +2 per iteration (each matvec)
        dve_sem = sem("dve_sem")    # +2 per iteration (each reciprocal)
        poolA = sem("poolA")        # +1 per fold (stage A done)
        peT = sem("peT")            # +1 per fold (psi transpose done)
        actP = sem("actP")         # +1 per fold (psi row copy done)
        poolB = sem("poolB")        # +1 per fold (T1 ready)
        actA2 = sem("actA2")        # +1 per fold (A2 regen done)
        poolX = sem("poolX")        # +1 per fold (X ready)
        peXT = sem("peXT")          # +1 per fold (X transpose done)
        actA1 = sem("actA1")        # +1 per fold (A1 regen done)
        poolSW = sem("poolSW")      # +1 per fold (A1sw ready)
        poolF = sem("poolF")        # final: Pu ready
        peF = sem("peF")            # final: P transpose ready
        dveF2 = sem("dveF2")        # final: PT7 ready

        lbc = W7[:, L:L + 1]
        invb = W7[:, L + 1:L + 2]
        bcol = W7[:, L + 2:L + 3]
        psi1 = W7[:, L + 3:L + 4]
        K = WK[:, B:2 * B]
        la = WK[:, 14:15]
        inva = WK[:, 15:16]
        A1 = [W7[:, 0:L], A1_1[:, :]]
        A2 = [WK[:, 0:B], A2_1[:, :]]
        pa = [pa_0[:, :], WK[:, 16:17]]
        psi_c = [psiC0[:, :], psi1]
        q = [q0t[:, :], q1t[:, :]]
        up = [u0t[:, :], u1t[:, :]]

        with nc.Block() as block:

            @block.sync
            def _(sync):
                nc.sync.dma_start(out=W7[:, :], in_=d_W7).then_inc(dsem, 16)
                nc.sync.dma_start(out=ident[:, :], in_=d_id).then_inc(dsemI, 16)
                with nc.allow_non_contiguous_dma(
                        reason="transposed 64x7 output, 1.8KB total"):
                    nc.sync.dma_start(
                        out=d_P.rearrange("a b -> b a"),
                        in_=PT7[:, :])._wait_ge(dveF2, 1).then_inc(dsem, 16)

            @block.tensor
            def _(te):
                epoch = 0
                for k, it in enumerate(iters, 1):
                    switching = it in switch_of
                    lhs1 = A1sw[:, :] if switching else A1[epoch]
                    lhs2 = A2[1 - epoch] if switching else A2[epoch]
                    q_in = invb if it == 2 else q[(it - 1) % 2]
                    if switching:
                        nc.tensor.wait_ge(poolSW, switch_of[it] + 1)
                    m1 = nc.tensor.matmul(psr[:, :], lhs1, q_in,
                                          start=True, stop=True)
                    if k > 1:
                        m1._wait_ge(dve_sem, 2 * (k - 1))
                    else:
                        m1._wait_ge(dsem, 16)
                    m1.then_inc(pe_sem)
                    if k == 1:
                        nc.tensor.wait_ge(dsem2, 16)
                    m2 = nc.tensor.matmul(psc[:, :], lhs2, up[it % 2],
                                          start=True, stop=True)
                    m2._wait_ge(dve_sem, 2 * k - 1)
                    m2.then_inc(pe_sem)
                    if switching:
                        epoch = 1 - epoch
                    if it in psiT_slot:
                        f = psiT_slot[it]
                        fp = f % 2
                        if f == 0:
                            nc.tensor.wait_ge(dsemI, 16)   # ident DMA
                        else:
                            nc.tensor.wait_ge(actP, f)     # WAR: psq reuse
                        tp = nc.tensor.transpose(psq[:, :], psi_c[fp],
                                                 ident[0:B, 0:B])
                        tp._wait_ge(poolA, f + 1)
                        tp.then_inc(peT)
                    if it in xt_slot:
                        f = xt_slot[it]
                        if f > 0:
                            nc.tensor.wait_ge(actA1, f)    # WAR: psa reuse
                        t1 = nc.tensor.transpose(psa[:, :], X[:, :],
                                                 ident[:, :])
                        t1._wait_ge(poolX, f + 1)
                        t1.then_inc(peXT)
                # final: transpose Pu into psa
                nc.tensor.wait_ge(actA1, n_folds)   # WAR: psa vs last A1 copy
                tf = nc.tensor.transpose(psa[:, :], Pu[:, :], ident[:, :])
                tf._wait_ge(poolF, 1)
                tf.then_inc(peF)

            @block.vector
            def _(v):
                for k, it in enumerate(iters, 1):
                    par = it % 2
                    if it in war_slot:
                        nc.vector.wait_ge(poolA, war_slot[it] + 1)
                    r1 = nc.vector.reciprocal(up[par], psr[:, :])
                    r1._wait_ge(pe_sem, 2 * k - 1)
                    r1.then_inc(dve_sem)
                    r2 = nc.vector.reciprocal(q[par], psc[:, :])
                    r2._wait_ge(pe_sem, 2 * k)
                    r2.then_inc(dve_sem)
                # final: bq = q * b ; PT7 = psa * bq
                bqi = nc.vector.tensor_scalar(out=bq[:, :], in0=q[final_par],
                                              scalar1=bcol, scalar2=None,
                                              op0=OP.mult)
                bqi._wait_ge(dve_sem, 2 * k_last)
                pt = nc.vector.tensor_scalar(out=PT7[:, :], in0=psa[:, :],
                                             scalar1=bq[:, :], scalar2=None,
                                             op0=OP.mult)
                pt._wait_ge(peF, 1)
                pt.then_inc(dveF2)

            @block.scalar
            def _(s):
                nc.scalar.activation(scr7[:, :], lbc, AF.Exp)._wait_ge(dsem, 16)
                for f, it_f in enumerate(folds):
                    fp = f % 2
                    ne = 1 - (f % 2)
                    cbx = nc.scalar.activation(cb[:, :], ncb[:, :], AF.Exp)
                    cbx._wait_ge(poolA, f + 1)
                    pr = nc.scalar.copy(psi_r[:, :], psq[:, :])
                    pr._wait_ge(peT, f + 1)
                    pr.then_inc(actP)
                    a2x = nc.scalar.activation(A2[ne], T1[:, :], AF.Exp,
                                               bias=pa[fp])
                    a2x._wait_ge(poolB, f + 1)
                    a2x.then_inc(actA2)
                    a1c = nc.scalar.activation(A1[ne], psa[:, :], AF.Copy,
                                               scale=bcol)
                    a1c._wait_ge(peXT, f + 1)
                    a1c.then_inc(actA1)

            @block.gpsimd
            def _(g):
                nc.gpsimd.dma_start(out=WK[:, :], in_=d_WK).then_inc(dsem2, 16)
                for f, it_f in enumerate(folds):
                    par = it_f % 2
                    fp = f % 2
                    ne = 1 - (f % 2)
                    g1 = nc.gpsimd.tensor_copy(yfu[:, :], up[par].bitcast(u32))
                    g1._wait_ge(dve_sem, 2 * (it_f - 1))
                    nc.gpsimd.tensor_scalar(out=lnu[:, :], in0=yfu[:, :],
                                            scalar1=C1, scalar2=C2,
                                            op0=OP.mult, op1=OP.add)
                    nc.gpsimd.tensor_copy(yfq[:, :], q[par].bitcast(u32))
                    nc.gpsimd.tensor_scalar(out=lnq[:, :], in0=yfq[:, :],
                                            scalar1=C1, scalar2=C2,
                                            op0=OP.mult, op1=OP.add)
                    nc.gpsimd.tensor_scalar(out=pa[fp], in0=lnu[:, :],
                                            scalar1=pa[1 - fp], scalar2=la,
                                            op0=OP.add, op1=OP.add)
                    nc.gpsimd.tensor_scalar(out=psi_c[fp], in0=lnq[:, :],
                                            scalar1=psi_c[1 - fp], scalar2=lbc,
                                            op0=OP.add, op1=OP.add)
                    nc.gpsimd.tensor_scalar(out=ncb[:, :], in0=lnq[:, :],
                                            scalar1=lbc, scalar2=-1.0,
                                            op0=OP.add,
                                            op1=OP.mult).then_inc(poolA)
                    pb = nc.gpsimd.partition_broadcast(psi_bc[:, :],
                                                       psi_r[:, :])
                    pb._wait_ge(actP, f + 1)
                    nc.gpsimd.tensor_tensor(out=T1[:, :], in0=K,
                                            in1=psi_bc[:, :],
                                            op=OP.add).then_inc(poolB)
                    xx = nc.gpsimd.tensor_scalar(out=X[:, :], in0=A2[ne],
                                                 scalar1=inva, scalar2=None,
                                                 op0=OP.mult)
                    xx._wait_ge(actA2, f + 1)
                    xx.then_inc(poolX)
                    sw = nc.gpsimd.tensor_scalar(out=A1sw[:, :], in0=A1[ne],
                                                 scalar1=cb[:, :], scalar2=None,
                                                 op0=OP.mult)
                    sw._wait_ge(actA1, f + 1)
                    sw.then_inc(poolSW)
                # final: Pu = A2 * u
                pu = nc.gpsimd.tensor_scalar(out=Pu[:, :], in0=A2[final_epoch],
                                             scalar1=up[final_par],
                                             scalar2=None, op0=OP.mult)
                pu._wait_ge(dve_sem, 2 * k_last)
                pu.then_inc(poolF)

        nc.compile()
    return nc


def _host_inputs(theta, phi, n, sens, err):
    f32 = np.float32
    theta = np.asarray(theta, f32); phi = np.asarray(phi, f32)
    n = np.asarray(n, f32); sens = np.asarray(sens, f32)
    err = np.asarray(err, f32)
    a = (n / n.sum()).astype(f32)
    e = np.exp((phi - phi.max()).astype(f32)); b = (e / e.sum()).astype(f32)
    C = ((n * sens)[:, None] * err[None, :]).astype(f32)
    K = ((theta - C) * f32(1.0 / EPS)).astype(f32)
    la = np.log(a).astype(f32)
    lb = np.log(b).astype(f32)

    # iteration 1 (log domain, max-stabilized LSE) + initial basis, on host
    def lse(x, axis):
        m = x.max(axis=axis, keepdims=True)
        return (m + np.log(np.exp(x - m).sum(axis=axis, keepdims=True))
                ).squeeze(axis).astype(f32)

    def ftz(x):
        x = np.asarray(x, f32).copy()
        x[np.abs(x) < 1.17549435e-38] = 0.0
        return x

    f1 = (la - lse(K, 1)).astype(f32)
    g1 = (lb - lse(K + f1[:, None], 0)).astype(f32)
    pa0 = (f1 + la).astype(f32)
    A2_0 = ftz(np.exp((K + pa0[:, None] + g1[None, :]).astype(f32)))
    A1_0 = ftz(ftz(A2_0 * (f32(1.0) / a)[:, None]).T * b[:, None])
    inva = (f32(1.0) / a).astype(f32)

    W7 = np.concatenate(
        [A1_0, np.stack([lb, f32(1.0) / b, b, g1], axis=1)], axis=1).astype(f32)
    WK = np.concatenate(
        [A2_0, K, np.stack([la, inva, pa0], axis=1)], axis=1).astype(f32)
    return {
        "W7_in": np.ascontiguousarray(W7),
        "W64K_in": np.ascontiguousarray(WK),
        "ident_in": np.eye(L, dtype=f32),
    }


def kernel(theta, phi, n, sens, err):
    if "nc" not in _CACHE:
        _CACHE["nc"] = _build_nc()
    nc = _CACHE["nc"]
    in_map = _host_inputs(theta, phi, n, sens, err)
    from concourse import bass_utils
    res = bass_utils.run_bass_kernel_spmd(nc, [in_map], [0])
    _CACHE["res"] = res
    return np.asarray(res.results[0]["P_out"], dtype=np.float32)


# revision 24
# speedup vs baseline: 1.5500x; 1.0220x over previous
"""Bass/Trainium2 kernel for nn_DiffAllocator (64x7 Sinkhorn, 200 iterations).

Raw-bass implementation (no Tile framework) with hand-rolled semaphores,
using the proven 4-op loop structure:

    r = A1 q   (PE matvec)   u = 1/r  (DVE reciprocal)
    c = A2 u   (PE matvec)   q = 1/c  (DVE reciprocal)

Iteration 1 (the only exact-LSE user) runs on host; iterations 2..200 run on
device in this multiplicative form anchored at stabilizers, re-anchored
("fold") every 16 iterations so no flushed-to-zero entry is ever remembered.
Every in-loop dependency is cross-engine (PE<->DVE semaphores) - the pattern
whose 536ns/iteration cost is hardware-validated. Same-engine DVE chaining is
NOT used (DVE SBUF writes drain ~60-125ns after the engine frees; a program-
order consumer races, as measured).

Why raw bass: the Tile framework routes fold work through the same per-engine
semaphore counters as the loop, so each fold's snapshot reads and PE
transposes stall the loop ~460ns (3 stalls/fold, ~5.6us total). With separate
semaphores per producer/consumer pair the folds run entirely in the loop's
shadow on Pool/ACT (+2 donated PE transpose slots), and the switch waits are
pre-satisfied. The it=192 fold is dropped (the 24-iteration tail drifts only
~e^30, far inside fp32 range) and the basis switch applies DELAY=8 iterations
after the snapshot (fp32-validated in emulation).

The output DMA instruction is issued up-front with its semaphore wait
attached, so the DGE setup overlaps the loop and only the transfer itself
lands in the tail.
"""

import numpy as np

L, B = 64, 7
EPS = 0.02
ITERS = 200
FOLD_EVERY = 16
LAST_FOLD = 176
DELAY = 9       # fold snapshot -> basis switch
PSIT_SLOT = 2   # fold + n: PE emits the psi transpose after this iteration
XT_SLOT = 6     # fold + n: PE emits the X transpose after this iteration

_CACHE = {}


def _build_nc(reps=1):
    import contextlib
    import concourse.bacc as bacc
    import concourse.mybir as mybir

    f32 = mybir.dt.float32
    u32 = mybir.dt.uint32
    AF = mybir.ActivationFunctionType
    OP = mybir.AluOpType

    nc = bacc.Bacc("TRN2", target_bir_lowering=False, debug=False)

    # ---- DRAM I/O ----
    # W7   = [A1_0 (64) | lb | 1/b | b | psi0]      -> [7, 68]
    # W64K = [A2_0 (7) | K (7) | la | 1/a | pa0]    -> [64, 17]
    d_W7 = nc.dram_tensor("W7_in", [B, L + 4], f32, kind="ExternalInput").ap()
    d_WK = nc.dram_tensor("W64K_in", [L, 17], f32, kind="ExternalInput").ap()
    d_id = nc.dram_tensor("ident_in", [L, L], f32, kind="ExternalInput").ap()
    d_P = nc.dram_tensor("P_out", [L, B], f32, kind="ExternalOutput").ap()

    n_end = 2 + (ITERS - 1) * reps
    iters = list(range(2, n_end))
    folds = [it for it in iters if it % FOLD_EVERY == 0 and it <= LAST_FOLD]
    fold_of = {it_f: f for f, it_f in enumerate(folds)}
    switch_of = {it_f + DELAY: f for f, it_f in enumerate(folds)}
    psiT_slot = {it_f + PSIT_SLOT: f for f, it_f in enumerate(folds)}
    xt_slot = {it_f + XT_SLOT: f for f, it_f in enumerate(folds)}
    war_slot = {it_f + 4: f for f, it_f in enumerate(folds)}
    n_folds = len(folds)
    final_epoch = n_folds % 2
    final_par = iters[-1] % 4
    k_last = len(iters)

    LN2 = float(np.log(2.0))
    C1, C2 = LN2 / (2.0 ** 23), -127.0 * LN2

    es = contextlib.ExitStack()
    with es:
        sb = lambda name, shape: es.enter_context(
            nc.sbuf_tensor(name, shape, f32))
        W7 = sb("W7", [B, L + 4])
        WK = sb("WK", [L, 17])
        ident = sb("ident", [L, L])
        A1_1 = sb("A1_1", [B, L])
        A2_1 = sb("A2_1", [L, B])
        A1sw = sb("A1sw", [B, L])
        qts = [sb(f"q{i}", [B, 1]) for i in range(4)]
        uts = [sb(f"u{i}", [L, 1]) for i in range(4)]
        pa_0 = sb("pa_0", [L, 1])
        psiC0 = sb("psiC0", [B, 1])
        psi_r = sb("psi_r", [1, B])
        psi_bc = sb("psi_bc", [L, B])
        T1 = sb("T1", [L, B])
        X = sb("X", [L, B])
        lnu = sb("lnu", [L, 1]); yfu = sb("yfu", [L, 1])
        lnq = sb("lnq", [B, 1]); yfq = sb("yfq", [B, 1])
        ncb = sb("ncb", [B, 1]); cb = sb("cb", [B, 1])
        Pu = sb("Pu", [L, B])
        bq = sb("bq", [B, 1])
        PT7 = sb("PT7", [B, L])
        scr7 = sb("scr7", [B, 1])

        psr = es.enter_context(nc.psum_tensor("psr", [L, 1], f32))
        psc = es.enter_context(nc.psum_tensor("psc", [B, 1], f32))
        psq = es.enter_context(nc.psum_tensor("psq", [1, B], f32))
        psa = es.enter_context(nc.psum_tensor("psa", [B, L], f32))

        sem = lambda name: es.enter_context(nc.semaphore(name))
        dsem = sem("dsem")          # W7 DMA
        dsemI = sem("dsemI")        # ident DMA
        dsem2 = sem("dsem2")        # W64K DMA (Pool queue)
        pe_sem = sem("pe_sem")      # +2 per iteration (each matvec)
        dve_sem = sem("dve_sem")    # +2 per iteration (each reciprocal)
        poolA = sem("poolA")        # +1 per fold (stage A done)
        peT = sem("peT")            # +1 per fold (psi transpose done)
        actP = sem("actP")         # +1 per fold (psi row copy done)
        poolB = sem("poolB")        # +1 per fold (T1 ready)
        actA2 = sem("actA2")        # +1 per fold (A2 regen done)
        poolX = sem("poolX")        # +1 per fold (X ready)
        peXT = sem("peXT")          # +1 per fold (X transpose done)
        actA1 = sem("actA1")        # +1 per fold (A1 regen done)
        poolSW = sem("poolSW")      # +1 per fold (A1sw ready)
        poolF = sem("poolF")        # final: Pu ready
        peF = sem("peF")            # final: P transpose ready
        dveF2 = sem("dveF2")        # final: PT7 ready

        lbc = W7[:, L:L + 1]
        invb = W7[:, L + 1:L + 2]
        bcol = W7[:, L + 2:L + 3]
        psi1 = W7[:, L + 3:L + 4]
        K = WK[:, B:2 * B]
        la = WK[:, 14:15]
        inva = WK[:, 15:16]
        A1 = [W7[:, 0:L], A1_1[:, :]]
        A2 = [WK[:, 0:B], A2_1[:, :]]
        pa = [pa_0[:, :], WK[:, 16:17]]
        psi_c = [psiC0[:, :], psi1]
        q = [t[:, :] for t in qts]
        up = [t[:, :] for t in uts]

        with nc.Block() as block:

            @block.sync
            def _(sync):
                nc.sync.dma_start(out=W7[:, :], in_=d_W7).then_inc(dsem, 16)
                nc.sync.dma_start(out=ident[:, :], in_=d_id).then_inc(dsemI, 16)
                with nc.allow_non_contiguous_dma(
                        reason="transposed 64x7 output, 1.8KB total"):
                    nc.sync.dma_start(
                        out=d_P.rearrange("a b -> b a"),
                        in_=PT7[:, :])._wait_ge(dveF2, 1).then_inc(dsem, 16)

            @block.tensor
            def _(te):
                epoch = 0
                for k, it in enumerate(iters, 1):
                    switching = it in switch_of
                    lhs1 = A1sw[:, :] if switching else A1[epoch]
                    lhs2 = A2[1 - epoch] if switching else A2[epoch]
                    q_in = invb if it == 2 else q[(it - 1) % 4]
                    if switching:
                        nc.tensor.wait_ge(poolSW, switch_of[it] + 1)
                    m1 = nc.tensor.matmul(psr[:, :], lhs1, q_in,
                                          start=True, stop=True)
                    if k > 1:
                        m1._wait_ge(dve_sem, 2 * (k - 1))
                    else:
                        m1._wait_ge(dsem, 16)
                    m1.then_inc(pe_sem)
                    if k == 1:
                        nc.tensor.wait_ge(dsem2, 16)
                    m2 = nc.tensor.matmul(psc[:, :], lhs2, up[it % 4],
                                          start=True, stop=True)
                    m2._wait_ge(dve_sem, 2 * k - 1)
                    m2.then_inc(pe_sem)
                    if switching:
                        epoch = 1 - epoch
                    if it in psiT_slot:
                        f = psiT_slot[it]
                        fp = f % 2
                        if f == 0:
                            nc.tensor.wait_ge(dsemI, 16)   # ident DMA
                        else:
                            nc.tensor.wait_ge(actP, f)     # WAR: psq reuse
                        tp = nc.tensor.transpose(psq[:, :], psi_c[fp],
                                                 ident[0:B, 0:B])
                        tp._wait_ge(poolA, f + 1)
                        tp.then_inc(peT)
                    if it in xt_slot:
                        f = xt_slot[it]
                        if f > 0:
                            nc.tensor.wait_ge(actA1, f)    # WAR: psa reuse
                        t1 = nc.tensor.transpose(psa[:, :], X[:, :],
                                                 ident[:, :])
                        t1._wait_ge(poolX, f + 1)
                        t1.then_inc(peXT)
                # final: transpose Pu into psa
                nc.tensor.wait_ge(actA1, n_folds)   # WAR: psa vs last A1 copy
                tf = nc.tensor.transpose(psa[:, :], Pu[:, :], ident[:, :])
                tf._wait_ge(poolF, 1)
                tf.then_inc(peF)

            @block.vector
            def _(v):
                for k, it in enumerate(iters, 1):
                    par = it % 4
                    if it in war_slot:
                        nc.vector.wait_ge(poolA, war_slot[it] + 1)
                    r1 = nc.vector.reciprocal(up[par], psr[:, :])
                    r1._wait_ge(pe_sem, 2 * k - 1)
                    r1.then_inc(dve_sem)
                    r2 = nc.vector.reciprocal(q[par], psc[:, :])
                    r2._wait_ge(pe_sem, 2 * k)
                    r2.then_inc(dve_sem)
                # final: bq = q * b ; PT7 = psa * bq
                bqi = nc.vector.tensor_scalar(out=bq[:, :], in0=q[final_par],
                                              scalar1=bcol, scalar2=None,
                                              op0=OP.mult)
                bqi._wait_ge(dve_sem, 2 * k_last)
                pt = nc.vector.tensor_scalar(out=PT7[:, :], in0=psa[:, :],
                                             scalar1=bq[:, :], scalar2=None,
                                             op0=OP.mult)
                pt._wait_ge(peF, 1)
                pt.then_inc(dveF2)

            @block.scalar
            def _(s):
                nc.scalar.activation(scr7[:, :], lbc, AF.Exp)._wait_ge(dsem, 16)
                for f, it_f in enumerate(folds):
                    fp = f % 2
                    ne = 1 - (f % 2)
                    cbx = nc.scalar.activation(cb[:, :], ncb[:, :], AF.Exp)
                    cbx._wait_ge(poolA, f + 1)
                    pr = nc.scalar.copy(psi_r[:, :], psq[:, :])
                    pr._wait_ge(peT, f + 1)
                    pr.then_inc(actP)
                    a2x = nc.scalar.activation(A2[ne], T1[:, :], AF.Exp,
                                               bias=pa[fp])
                    a2x._wait_ge(poolB, f + 1)
                    a2x.then_inc(actA2)
                    a1c = nc.scalar.activation(A1[ne], psa[:, :], AF.Copy,
                                               scale=bcol)
                    a1c._wait_ge(peXT, f + 1)
                    a1c.then_inc(actA1)

            @block.gpsimd
            def _(g):
                nc.gpsimd.dma_start(out=WK[:, :], in_=d_WK).then_inc(dsem2, 16)
                for f, it_f in enumerate(folds):
                    par = it_f % 4
                    fp = f % 2
                    ne = 1 - (f % 2)
                    g1 = nc.gpsimd.tensor_copy(yfu[:, :], up[par].bitcast(u32))
                    g1._wait_ge(dve_sem, 2 * (it_f - 1))
                    nc.gpsimd.tensor_scalar(out=lnu[:, :], in0=yfu[:, :],
                                            scalar1=C1, scalar2=C2,
                                            op0=OP.mult, op1=OP.add)
                    nc.gpsimd.tensor_copy(yfq[:, :], q[par].bitcast(u32))
                    nc.gpsimd.tensor_scalar(out=lnq[:, :], in0=yfq[:, :],
                                            scalar1=C1, scalar2=C2,
                                            op0=OP.mult, op1=OP.add)
                    nc.gpsimd.tensor_scalar(out=pa[fp], in0=lnu[:, :],
                                            scalar1=pa[1 - fp], scalar2=la,
                                            op0=OP.add, op1=OP.add)
                    nc.gpsimd.tensor_scalar(out=psi_c[fp], in0=lnq[:, :],
                                            scalar1=psi_c[1 - fp], scalar2=lbc,
                                            op0=OP.add, op1=OP.add)
                    nc.gpsimd.tensor_scalar(out=ncb[:, :], in0=lnq[:, :],
                                            scalar1=lbc, scalar2=-1.0,
                                            op0=OP.add,
                                            op1=OP.mult).then_inc(poolA)
                    pb = nc.gpsimd.partition_broadcast(psi_bc[:, :],
                                                       psi_r[:, :])
                    pb._wait_ge(actP, f + 1)
                    nc.gpsimd.tensor_tensor(out=T1[:, :], in0=K,
                                            in1=psi_bc[:, :],
                                            op=OP.add).then_inc(poolB)
                    xx = nc.gpsimd.tensor_scalar(out=X[:, :], in0=A2[ne],
                                                 scalar1=inva, scalar2=None,
                                                 op0=OP.mult)
                    xx._wait_ge(actA2, f + 1)
                    xx.then_inc(poolX)
                    sw = nc.gpsimd.tensor_scalar(out=A1sw[:, :], in0=A1[ne],
                                                 scalar1=cb[:, :], scalar2=None,
                                                 op0=OP.mult)
                    sw._wait_ge(actA1, f + 1)
                    sw.then_inc(poolSW)
                # final: Pu = A2 * u
                pu = nc.gpsimd.tensor_scalar(out=Pu[:, :], in0=A2[final_epoch],
                                             scalar1=up[final_par],
                                             scalar2=None, op0=OP.mult)
                pu._wait_ge(dve_sem, 2 * k_last - 1)
                pu.then_inc(poolF)

        nc.compile()
    return nc


def _host_inputs(theta, phi, n, sens, err):
    f32 = np.float32
    theta = np.asarray(theta, f32); phi = np.asarray(phi, f32)
    n = np.asarray(n, f32); sens = np.asarray(sens, f32)
    err = np.asarray(err, f32)
    a = (n / n.sum()).astype(f32)
    e = np.exp((phi - phi.max()).astype(f32)); b = (e / e.sum()).astype(f32)
    C = ((n * sens)[:, None] * err[None, :]).astype(f32)
    K = ((theta - C) * f32(1.0 / EPS)).astype(f32)
    la = np.log(a).astype(f32)
    lb = np.log(b).astype(f32)

    # iteration 1 (log domain, max-stabilized LSE) + initial basis, on host
    def lse(x, axis):
        m = x.max(axis=axis, keepdims=True)
        return (m + np.log(np.exp(x - m).sum(axis=axis, keepdims=True))
                ).squeeze(axis).astype(f32)

    def ftz(x):
        x = np.asarray(x, f32).copy()
        x[np.abs(x) < 1.17549435e-38] = 0.0
        return x

    f1 = (la - lse(K, 1)).astype(f32)
    g1 = (lb - lse(K + f1[:, None], 0)).astype(f32)
    pa0 = (f1 + la).astype(f32)
    A2_0 = ftz(np.exp((K + pa0[:, None] + g1[None, :]).astype(f32)))
    A1_0 = ftz(ftz(A2_0 * (f32(1.0) / a)[:, None]).T * b[:, None])
    inva = (f32(1.0) / a).astype(f32)

    W7 = np.concatenate(
        [A1_0, np.stack([lb, f32(1.0) / b, b, g1], axis=1)], axis=1).astype(f32)
    WK = np.concatenate(
        [A2_0, K, np.stack([la, inva, pa0], axis=1)], axis=1).astype(f32)
    return {
        "W7_in": np.ascontiguousarray(W7),
        "W64K_in": np.ascontiguousarray(WK),
        "ident_in": np.eye(L, dtype=f32),
    }


def kernel(theta, phi, n, sens, err):
    if "nc" not in _CACHE:
        _CACHE["nc"] = _build_nc()
    nc = _CACHE["nc"]
    in_map = _host_inputs(theta, phi, n, sens, err)
    from concourse import bass_utils
    res = bass_utils.run_bass_kernel_spmd(nc, [in_map], [0])
    _CACHE["res"] = res
    return np.asarray(res.results[0]["P_out"], dtype=np.float32)
